# revision 17
# baseline (speedup 1.0000x reference)
"""Trainium2 kernel: segment-mean aggregation (nn_Aggregate).

Computes, for S = batch_size * n_nodes segments:
    out[s // N, s % N, :] = mean of edge_vec rows whose selected_edges[:,5] == s

Strategy (8 NeuronCores, SPMD, no collectives):
  * Host shards edges by DESTINATION segment: segments are assigned to cores
    (contiguous S/8 ranges) and, within a core, bin-packed into NB blocks of
    <=64 segments and <= K*128 edges (fp16 edge stream, ~99% slot fill).
  * Device, per block: build a one-hot matrix onehot[e, s] = (seg_rel[e] == s)
    with one fused DVE is_equal op per super-block, then accumulate
    sums[s, d] += onehot.T @ vec with K matmuls into PSUM.  Two 64-segment
    blocks stack on PSUM partitions (base_partition 0 / 64), so the divide-
    by-count epilogue (Scalar-engine activation with per-partition scale)
    and the output DMA run at full 128-partition width.
  * Host inverse-permutes the per-core output rows into the [B, N, D] grid.

All floating-point reduction work (the ~1 GB of edge summation) happens on
device; the host only computes integer index metadata (bincount/argsort) and
performs the shard permutation implied by the sharding strategy.
"""

import os
import sys

import numpy as np

for _p in ("/opt/trn_rl_repo", "/root/.axon_site/_ro/trn_rl_repo"):
    if os.path.isdir(_p) and _p not in sys.path:
        sys.path.append(_p)

# Problem constants (hardcoded per spec nn_Aggregate_8985071583847)
E = 2_000_000
D = 128
B = 16
N = 20_000
S = B * N
NCORES = 8

# Kernel tiling parameters
G = 64       # segment slots per block (two blocks stack on 128 PSUM partitions)
K = 3        # edge tiles (of 128) per block
BPS = 16     # blocks per super-block (DMA batching granularity; 8 PSUM pairs)
PAD_SEGREL = 999.0  # exact in fp16; never matches iota in [0, G)


def _pack_segments(counts2, NB, cap):
    """Bin-pack each core's segments into NB blocks of <= G segments and
    <= cap edges: sort by count descending, assign each segment to the
    least-loaded (by edges) block with free segment slots (LPT).

    counts2: [ncores, seg_per_core] per-segment edge counts.
    Returns (ok, binid, slocal, estart) each [ncores, seg_per_core].
    """
    import heapq

    ncores, spc = counts2.shape
    if spc > NB * G:
        return False, None, None, None
    binid = np.empty((ncores, spc), np.int32)
    slocal = np.empty((ncores, spc), np.int32)
    estart = np.empty((ncores, spc), np.int64)
    for c in range(ncores):
        cc = counts2[c]
        order = np.argsort(-cc, kind="stable")
        heap = [(0, b) for b in range(NB)]
        nsegs = np.zeros(NB, np.int32)
        loads = np.zeros(NB, np.int64)
        bid_c = binid[c]
        sl_c = slocal[c]
        es_c = estart[c]
        for s in order:
            cnt = cc[s]
            while True:
                load, b = heapq.heappop(heap)
                if nsegs[b] < G:
                    break
            bid_c[s] = b
            sl_c[s] = nsegs[b]
            es_c[s] = loads[b]
            loads[b] += cnt
            nsegs[b] += 1
            if nsegs[b] < G:
                heapq.heappush(heap, (int(loads[b]), b))
        if loads.max() > cap:
            return False, None, None, None
    return True, binid, slocal, estart


def _prepare(edge_vec, seg, s_total, ncores, k, bps):
    """Host-side sharding: returns per-core input arrays + unshard map."""
    e_total, d = edge_vec.shape
    spc = s_total // ncores
    cap = k * 128

    counts = np.bincount(seg, minlength=s_total).astype(np.int64)
    counts2 = counts.reshape(ncores, spc)

    nb = max(
        int(np.ceil(counts2.sum(1).max() / cap)),
        int(np.ceil(spc / G)),
    )
    nb = -(-nb // bps) * bps
    while True:
        ok, binid, slocal, estart = _pack_segments(counts2, nb, cap)
        if ok:
            break
        nb += bps

    binid = binid.ravel().astype(np.int64)
    slocal = slocal.ravel().astype(np.int64)
    estart = estart.ravel()
    core_s = np.arange(s_total, dtype=np.int64) // spc

    order_e = np.argsort(seg, kind="stable")
    seg_sorted = seg[order_e]
    seg_start = np.zeros(s_total + 1, np.int64)
    np.cumsum(counts, out=seg_start[1:])
    within = np.arange(e_total, dtype=np.int64) - seg_start[seg_sorted]

    slot = estart[seg_sorted] + within          # edge slot within block [0, cap)
    t_e = slot // 128
    p_e = slot % 128
    b_e = binid[seg_sorted]
    c_e = core_s[seg_sorted]

    nbrows = nb * k * 128                        # stream rows per core
    row = ((b_e // bps) * 128 + p_e) * (bps * k) + (b_e % bps) * k + t_e
    grow = c_e * nbrows + row

    vec16 = np.ascontiguousarray(edge_vec, dtype=np.float16)
    stream = np.zeros((ncores * nbrows, d), np.float16)
    stream[grow] = vec16[order_e]

    # seg_rel table: [ncores, 128(part), nb*k]; PAD for unused edge slots.
    segrel = np.full(ncores * 128 * nb * k, PAD_SEGREL, np.float16)
    segrel[(c_e * 128 + p_e) * (nb * k) + b_e * k + t_e] = slocal[seg_sorted]
    segrel = segrel.reshape(ncores, 128, nb * k)

    # 1/count per (core, psum partition, block pair); 1.0 for empty slots.
    # Partition row = slocal + 64 * (block parity).
    npair = nb // 2
    invc = np.ones(ncores * 128 * npair, np.float32)
    invc[
        (core_s * 128 + slocal + G * (binid % 2)) * npair + binid // 2
    ] = 1.0 / np.maximum(counts, 1)
    invc = invc.reshape(ncores, 128, npair)

    iota = np.broadcast_to(np.arange(G, dtype=np.float16), (128, G)).copy()

    # Inverse map into the partition-major device output layout
    # [nsb, 128, bps//2, d]: segment (binid, slocal) lives at super-block
    # isb = binid//bps, partition (binid%2)*G + slocal, pair j = (binid%bps)//2.
    isb_s = binid // bps
    j_s = (binid % bps) // 2
    q_s = binid % 2
    seg_of = np.full(ncores * nb * G, -1, np.int64)
    seg_of[
        core_s * (nb * G)
        + ((isb_s * 128 + q_s * G + slocal) * (bps // 2) + j_s)
    ] = np.arange(s_total)
    seg_of = seg_of.reshape(ncores, nb * G)

    return nb, stream, segrel, invc, iota, seg_of


def _build_graph(nb, k, bps, d):
    import concourse.tile as tile
    from concourse import bacc, mybir

    f16 = mybir.dt.float16
    f32 = mybir.dt.float32
    nsb = nb // bps
    bk = bps * k            # vec tiles per super-block (24)
    npairs = bps // 2       # psum column groups per super-block (4)

    nc = bacc.Bacc()
    vec_p = nc.declare_dram_parameter("vec", [nsb * 128, bk * 128], f16, isOutput=False)
    srel_p = nc.declare_dram_parameter("srel", [128, nb * k], f16, isOutput=False)
    invc_p = nc.declare_dram_parameter("invc", [128, nb // 2], f32, isOutput=False)
    iota_p = nc.declare_dram_parameter("iota", [128, G], f16, isOutput=False)
    out_p = nc.declare_dram_parameter(
        "out", [nsb * 128, npairs * d], f16, isOutput=True
    )

    with tile.TileContext(nc) as tc:
        with tc.tile_pool(name="const", bufs=1) as cpool, \
             tc.tile_pool(name="vecp", bufs=4) as vpool, \
             tc.tile_pool(name="ohp", bufs=4) as opool, \
             tc.tile_pool(name="resp", bufs=4) as rpool, \
             tc.tile_pool(name="psp", bufs=4, space="PSUM") as ppool:

            srel_t = cpool.tile([128, nb * k], f16)
            nc.sync.dma_start(out=srel_t[:], in_=srel_p[:, :])
            invc_t = cpool.tile([128, nb // 2], f32)
            nc.sync.dma_start(out=invc_t[:], in_=invc_p[:, :])
            iota_t = cpool.tile([128, G], f16)
            nc.sync.dma_start(out=iota_t[:], in_=iota_p[:, :])

            for isb in range(nsb):
                vt = vpool.tile([128, bk * 128], f16)
                dma_eng = nc.sync if isb % 2 == 0 else nc.scalar
                dma_eng.dma_start(
                    out=vt[:], in_=vec_p[isb * 128 : (isb + 1) * 128, :]
                )
                oh = opool.tile([128, bk * G], f16)
                nc.vector.tensor_tensor(
                    out=oh[:].rearrange("p (a s) -> p a s", s=G),
                    in0=srel_t[:, isb * bk : (isb + 1) * bk]
                    .unsqueeze(2)
                    .to_broadcast([128, bk, G]),
                    in1=iota_t[:].unsqueeze(1).to_broadcast([128, bk, G]),
                    op=mybir.AluOpType.is_equal,
                )
                ps = ppool.tile([128, npairs * 128], f32, space="PSUM")
                for j in range(npairs):
                    for q in range(2):
                        for t in range(k):
                            tau = (j * 2 + q) * k + t
                            nc.tensor.matmul(
                                out=ps[q * G : (q + 1) * G, j * 128 : (j + 1) * 128],
                                lhsT=oh[:, tau * G : (tau + 1) * G],
                                rhs=vt[:, tau * 128 : (tau + 1) * 128],
                                start=(t == 0),
                                stop=(t == k - 1),
                            )
                ot = rpool.tile([128, npairs * d], f16)
                nc.vector.tensor_tensor(
                    out=ot[:].rearrange("p (a s) -> p a s", s=d),
                    in0=ps[:].rearrange("p (a s) -> p a s", s=128),
                    in1=invc_t[:, isb * npairs : (isb + 1) * npairs]
                    .unsqueeze(2)
                    .to_broadcast([128, npairs, 128]),
                    op=mybir.AluOpType.mult,
                )
                nc.scalar.dma_start(
                    out=out_p[isb * 128 : (isb + 1) * 128, :],
                    in_=ot[:],
                )
    nc.compile()
    return nc


def _run(edge_vec, seg, s_total=S, ncores=NCORES, k=K, bps=BPS, trace=False):
    from concourse.bass_utils import run_bass_kernel_spmd

    edge_vec = np.asarray(edge_vec, dtype=np.float32)
    seg = np.asarray(seg, dtype=np.int64)
    d = edge_vec.shape[1]

    nb, stream, segrel, invc, iota, seg_of = _prepare(
        edge_vec, seg, s_total, ncores, k, bps
    )
    nc = _build_graph(nb, k, bps, d)

    nbrows = nb * k * 128
    in_maps = [
        {
            "vec": stream[c * nbrows : (c + 1) * nbrows].reshape(
                (nb // bps) * 128, bps * k * 128
            ),
            "srel": segrel[c],
            "invc": invc[c],
            "iota": iota,
        }
        for c in range(ncores)
    ]
    res = run_bass_kernel_spmd(
        nc, in_maps, core_ids=list(range(ncores)), trace=trace
    )

    dev = np.concatenate(
        [res.results[c]["out"].reshape(-1, d) for c in range(ncores)], axis=0
    )
    out_flat = np.zeros((s_total, d), np.float32)
    mask = seg_of.ravel() >= 0
    out_flat[seg_of.ravel()[mask]] = dev[mask].astype(np.float32)
    return out_flat, res, nc


def kernel(edge_vec, selected_edges, num_segments=S, batch_size=B, n_nodes=N):
    selected_edges = np.asarray(selected_edges)
    seg = np.asarray(selected_edges[:, 5], dtype=np.int64)
    s_total = int(num_segments)
    out_flat, _, _ = _run(edge_vec, seg, s_total=s_total)
    return out_flat.reshape(int(batch_size), int(n_nodes), -1)


# revision 18
# speedup vs baseline: 1.0477x; 1.0477x over previous
"""Trainium2 kernel: segment-mean aggregation (nn_Aggregate).

Computes, for S = batch_size * n_nodes segments:
    out[s // N, s % N, :] = mean of edge_vec rows whose selected_edges[:,5] == s

Strategy (8 NeuronCores, SPMD, no collectives):
  * Host shards edges by DESTINATION segment: segments are assigned to cores
    (contiguous S/8 ranges) and, within a core, bin-packed into NB blocks of
    <=64 segments and <= K*128 edges (fp16 edge stream, ~99% slot fill).
  * Device, per block: build a one-hot matrix onehot[e, s] = (seg_rel[e] == s)
    with one fused DVE is_equal op per super-block, then accumulate
    sums[s, d] += onehot.T @ vec with K matmuls into PSUM.  Two 64-segment
    blocks stack on PSUM partitions (base_partition 0 / 64), so the divide-
    by-count epilogue (Scalar-engine activation with per-partition scale)
    and the output DMA run at full 128-partition width.
  * Host inverse-permutes the per-core output rows into the [B, N, D] grid.

All floating-point reduction work (the ~1 GB of edge summation) happens on
device; the host only computes integer index metadata (bincount/argsort) and
performs the shard permutation implied by the sharding strategy.
"""

import os
import sys

import numpy as np

for _p in ("/opt/trn_rl_repo", "/root/.axon_site/_ro/trn_rl_repo"):
    if os.path.isdir(_p) and _p not in sys.path:
        sys.path.append(_p)

# Problem constants (hardcoded per spec nn_Aggregate_8985071583847)
E = 2_000_000
D = 128
B = 16
N = 20_000
S = B * N
NCORES = 8

# Kernel tiling parameters
G = 64       # segment slots per block (two blocks stack on 128 PSUM partitions)
K = 3        # edge tiles (of 128) per block
BPS = 16     # blocks per super-block (DMA batching granularity; 8 PSUM pairs)
PAD_SEGREL = 999.0  # exact in fp16; never matches iota in [0, G)


def _pack_segments(counts2, NB, cap):
    """Bin-pack each core's segments into NB blocks of <= G segments and
    <= cap edges: sort by count descending, assign each segment to the
    least-loaded (by edges) block with free segment slots (LPT).

    counts2: [ncores, seg_per_core] per-segment edge counts.
    Returns (ok, binid, slocal, estart) each [ncores, seg_per_core].
    """
    import heapq

    ncores, spc = counts2.shape
    if spc > NB * G:
        return False, None, None, None
    binid = np.empty((ncores, spc), np.int32)
    slocal = np.empty((ncores, spc), np.int32)
    estart = np.empty((ncores, spc), np.int64)
    for c in range(ncores):
        cc = counts2[c]
        order = np.argsort(-cc, kind="stable")
        heap = [(0, b) for b in range(NB)]
        nsegs = np.zeros(NB, np.int32)
        loads = np.zeros(NB, np.int64)
        bid_c = binid[c]
        sl_c = slocal[c]
        es_c = estart[c]
        for s in order:
            cnt = cc[s]
            while True:
                load, b = heapq.heappop(heap)
                if nsegs[b] < G:
                    break
            bid_c[s] = b
            sl_c[s] = nsegs[b]
            es_c[s] = loads[b]
            loads[b] += cnt
            nsegs[b] += 1
            if nsegs[b] < G:
                heapq.heappush(heap, (int(loads[b]), b))
        if loads.max() > cap:
            return False, None, None, None
    return True, binid, slocal, estart


def _prepare(edge_vec, seg, s_total, ncores, k, bps):
    """Host-side sharding: returns per-core input arrays + unshard map."""
    e_total, d = edge_vec.shape
    spc = s_total // ncores
    cap = k * 128

    counts = np.bincount(seg, minlength=s_total).astype(np.int64)
    counts2 = counts.reshape(ncores, spc)

    nb = max(
        int(np.ceil(counts2.sum(1).max() / cap)),
        int(np.ceil(spc / G)),
    )
    nb = -(-nb // bps) * bps
    while True:
        ok, binid, slocal, estart = _pack_segments(counts2, nb, cap)
        if ok:
            break
        nb += bps

    binid = binid.ravel().astype(np.int64)
    slocal = slocal.ravel().astype(np.int64)
    estart = estart.ravel()
    core_s = np.arange(s_total, dtype=np.int64) // spc

    order_e = np.argsort(seg, kind="stable")
    seg_sorted = seg[order_e]
    seg_start = np.zeros(s_total + 1, np.int64)
    np.cumsum(counts, out=seg_start[1:])
    within = np.arange(e_total, dtype=np.int64) - seg_start[seg_sorted]

    slot = estart[seg_sorted] + within          # edge slot within block [0, cap)
    t_e = slot // 128
    p_e = slot % 128
    b_e = binid[seg_sorted]
    c_e = core_s[seg_sorted]

    nbrows = nb * k * 128                        # stream rows per core
    row = ((b_e // bps) * 128 + p_e) * (bps * k) + (b_e % bps) * k + t_e
    grow = c_e * nbrows + row

    vec16 = np.ascontiguousarray(edge_vec, dtype=np.float16)
    stream = np.zeros((ncores * nbrows, d), np.float16)
    stream[grow] = vec16[order_e]

    # seg_rel table: [ncores, 128(part), nb*k]; PAD for unused edge slots.
    segrel = np.full(ncores * 128 * nb * k, PAD_SEGREL, np.float16)
    segrel[(c_e * 128 + p_e) * (nb * k) + b_e * k + t_e] = slocal[seg_sorted]
    segrel = segrel.reshape(ncores, 128, nb * k)

    # 1/count per (core, psum partition, block pair); 1.0 for empty slots.
    # Partition row = slocal + 64 * (block parity).
    npair = nb // 2
    invc = np.ones(ncores * 128 * npair, np.float32)
    invc[
        (core_s * 128 + slocal + G * (binid % 2)) * npair + binid // 2
    ] = 1.0 / np.maximum(counts, 1)
    invc = invc.reshape(ncores, 128, npair)

    iota = np.broadcast_to(np.arange(G, dtype=np.float16), (128, G)).copy()

    # Inverse map into the partition-major device output layout
    # [nsb, 128, bps//2, d]: segment (binid, slocal) lives at super-block
    # isb = binid//bps, partition (binid%2)*G + slocal, pair j = (binid%bps)//2.
    isb_s = binid // bps
    j_s = (binid % bps) // 2
    q_s = binid % 2
    seg_of = np.full(ncores * nb * G, -1, np.int64)
    seg_of[
        core_s * (nb * G)
        + ((isb_s * 128 + q_s * G + slocal) * (bps // 2) + j_s)
    ] = np.arange(s_total)
    seg_of = seg_of.reshape(ncores, nb * G)

    return nb, stream, segrel, invc, iota, seg_of


def _build_graph(nb, k, bps, d):
    import concourse.tile as tile
    from concourse import bacc, mybir

    f16 = mybir.dt.float16
    f32 = mybir.dt.float32
    nsb = nb // bps
    bk = bps * k            # vec tiles per super-block (24)
    npairs = bps // 2       # psum column groups per super-block (4)

    nc = bacc.Bacc()
    vec_p = nc.declare_dram_parameter("vec", [nsb * 128, bk * 128], f16, isOutput=False)
    srel_p = nc.declare_dram_parameter("srel", [128, nb * k], f16, isOutput=False)
    invc_p = nc.declare_dram_parameter("invc", [128, nb // 2], f32, isOutput=False)
    iota_p = nc.declare_dram_parameter("iota", [128, G], f16, isOutput=False)
    out_p = nc.declare_dram_parameter(
        "out", [nsb * 128, npairs * d], f16, isOutput=True
    )

    with tile.TileContext(nc) as tc:
        with tc.tile_pool(name="const", bufs=1) as cpool, \
             tc.tile_pool(name="vecp", bufs=5) as vpool, \
             tc.tile_pool(name="ohp", bufs=4) as opool, \
             tc.tile_pool(name="resp", bufs=4) as rpool, \
             tc.tile_pool(name="psp", bufs=4, space="PSUM") as ppool:

            srel_t = cpool.tile([128, nb * k], f16)
            nc.sync.dma_start(out=srel_t[:], in_=srel_p[:, :])
            invc_t = cpool.tile([128, nb // 2], f32)
            nc.sync.dma_start(out=invc_t[:], in_=invc_p[:, :])
            iota_t = cpool.tile([128, G], f16)
            nc.sync.dma_start(out=iota_t[:], in_=iota_p[:, :])

            for isb in range(nsb):
                vt = vpool.tile([128, bk * 128], f16)
                dma_eng = nc.sync if isb % 2 == 0 else nc.scalar
                dma_eng.dma_start(
                    out=vt[:], in_=vec_p[isb * 128 : (isb + 1) * 128, :]
                )
                oh = opool.tile([128, bk * G], f16)
                nc.vector.tensor_tensor(
                    out=oh[:].rearrange("p (a s) -> p a s", s=G),
                    in0=srel_t[:, isb * bk : (isb + 1) * bk]
                    .unsqueeze(2)
                    .to_broadcast([128, bk, G]),
                    in1=iota_t[:].unsqueeze(1).to_broadcast([128, bk, G]),
                    op=mybir.AluOpType.is_equal,
                )
                ps = ppool.tile([128, npairs * 128], f32, space="PSUM")
                for j in range(npairs):
                    for q in range(2):
                        for t in range(k):
                            tau = (j * 2 + q) * k + t
                            nc.tensor.matmul(
                                out=ps[q * G : (q + 1) * G, j * 128 : (j + 1) * 128],
                                lhsT=oh[:, tau * G : (tau + 1) * G],
                                rhs=vt[:, tau * 128 : (tau + 1) * 128],
                                start=(t == 0),
                                stop=(t == k - 1),
                            )
                ot = rpool.tile([128, npairs * d], f16)
                nc.vector.tensor_tensor(
                    out=ot[:].rearrange("p (a s) -> p a s", s=d),
                    in0=ps[:].rearrange("p (a s) -> p a s", s=128),
                    in1=invc_t[:, isb * npairs : (isb + 1) * npairs]
                    .unsqueeze(2)
                    .to_broadcast([128, npairs, 128]),
                    op=mybir.AluOpType.mult,
                )
                nc.scalar.dma_start(
                    out=out_p[isb * 128 : (isb + 1) * 128, :],
                    in_=ot[:],
                )
    nc.compile()
    return nc


def _run(edge_vec, seg, s_total=S, ncores=NCORES, k=K, bps=BPS, trace=False):
    from concourse.bass_utils import run_bass_kernel_spmd

    edge_vec = np.asarray(edge_vec, dtype=np.float32)
    seg = np.asarray(seg, dtype=np.int64)
    d = edge_vec.shape[1]

    nb, stream, segrel, invc, iota, seg_of = _prepare(
        edge_vec, seg, s_total, ncores, k, bps
    )
    nc = _build_graph(nb, k, bps, d)

    nbrows = nb * k * 128
    in_maps = [
        {
            "vec": stream[c * nbrows : (c + 1) * nbrows].reshape(
                (nb // bps) * 128, bps * k * 128
            ),
            "srel": segrel[c],
            "invc": invc[c],
            "iota": iota,
        }
        for c in range(ncores)
    ]
    res = run_bass_kernel_spmd(
        nc, in_maps, core_ids=list(range(ncores)), trace=trace
    )

    dev = np.concatenate(
        [res.results[c]["out"].reshape(-1, d) for c in range(ncores)], axis=0
    )
    out_flat = np.zeros((s_total, d), np.float32)
    mask = seg_of.ravel() >= 0
    out_flat[seg_of.ravel()[mask]] = dev[mask].astype(np.float32)
    return out_flat, res, nc


def kernel(edge_vec, selected_edges, num_segments=S, batch_size=B, n_nodes=N):
    selected_edges = np.asarray(selected_edges)
    seg = np.asarray(selected_edges[:, 5], dtype=np.int64)
    s_total = int(num_segments)
    out_flat, _, _ = _run(edge_vec, seg, s_total=s_total)
    return out_flat.reshape(int(batch_size), int(n_nodes), -1)


# revision 19
# speedup vs baseline: 1.1316x; 1.0801x over previous
"""Trainium2 kernel: segment-mean aggregation (nn_Aggregate).

Computes, for S = batch_size * n_nodes segments:
    out[s // N, s % N, :] = mean of edge_vec rows whose selected_edges[:,5] == s

Strategy (8 NeuronCores, SPMD, no collectives):
  * Host shards edges by DESTINATION segment: segments are assigned to cores
    (contiguous S/8 ranges) and, within a core, bin-packed into NB blocks of
    <=64 segments and <= K*128 edges (fp16 edge stream, ~99% slot fill).
  * Device, per block: build a one-hot matrix onehot[e, s] = (seg_rel[e] == s)
    with one fused DVE is_equal op per super-block, then accumulate
    sums[s, d] += onehot.T @ vec with K matmuls into PSUM.  Two 64-segment
    blocks stack on PSUM partitions (base_partition 0 / 64), so the divide-
    by-count epilogue (Scalar-engine activation with per-partition scale)
    and the output DMA run at full 128-partition width.
  * Host inverse-permutes the per-core output rows into the [B, N, D] grid.

All floating-point reduction work (the ~1 GB of edge summation) happens on
device; the host only computes integer index metadata (bincount/argsort) and
performs the shard permutation implied by the sharding strategy.
"""

import os
import sys

import numpy as np

for _p in ("/opt/trn_rl_repo", "/root/.axon_site/_ro/trn_rl_repo"):
    if os.path.isdir(_p) and _p not in sys.path:
        sys.path.append(_p)

# Problem constants (hardcoded per spec nn_Aggregate_8985071583847)
E = 2_000_000
D = 128
B = 16
N = 20_000
S = B * N
NCORES = 8

# Kernel tiling parameters
G = 64       # segment slots per block (two blocks stack on 128 PSUM partitions)
K = 3        # edge tiles (of 128) per block
BPS = 16     # blocks per super-block (DMA batching granularity; 8 PSUM pairs)
PAD_SEGREL = 999.0  # exact in fp16; never matches iota in [0, G)


def _pack_segments(counts2, NB, cap):
    """Bin-pack each core's segments into NB blocks of <= G segments and
    <= cap edges: sort by count descending, assign each segment to the
    least-loaded (by edges) block with free segment slots (LPT).

    counts2: [ncores, seg_per_core] per-segment edge counts.
    Returns (ok, binid, slocal, estart) each [ncores, seg_per_core].
    """
    import heapq

    ncores, spc = counts2.shape
    if spc > NB * G:
        return False, None, None, None
    binid = np.empty((ncores, spc), np.int32)
    slocal = np.empty((ncores, spc), np.int32)
    estart = np.empty((ncores, spc), np.int64)
    for c in range(ncores):
        cc = counts2[c]
        order = np.argsort(-cc, kind="stable")
        heap = [(0, b) for b in range(NB)]
        nsegs = np.zeros(NB, np.int32)
        loads = np.zeros(NB, np.int64)
        bid_c = binid[c]
        sl_c = slocal[c]
        es_c = estart[c]
        for s in order:
            cnt = cc[s]
            while True:
                load, b = heapq.heappop(heap)
                if nsegs[b] < G:
                    break
            bid_c[s] = b
            sl_c[s] = nsegs[b]
            es_c[s] = loads[b]
            loads[b] += cnt
            nsegs[b] += 1
            if nsegs[b] < G:
                heapq.heappush(heap, (int(loads[b]), b))
        if loads.max() > cap:
            return False, None, None, None
    return True, binid, slocal, estart


def _prepare(edge_vec, seg, s_total, ncores, k, bps):
    """Host-side sharding: returns per-core input arrays + unshard map."""
    e_total, d = edge_vec.shape
    spc = s_total // ncores
    cap = k * 128

    counts = np.bincount(seg, minlength=s_total).astype(np.int64)
    counts2 = counts.reshape(ncores, spc)

    nb = max(
        int(np.ceil(counts2.sum(1).max() / cap)),
        int(np.ceil(spc / G)),
    )
    nb = -(-nb // bps) * bps
    while True:
        ok, binid, slocal, estart = _pack_segments(counts2, nb, cap)
        if ok:
            break
        nb += bps

    binid = binid.ravel().astype(np.int64)
    slocal = slocal.ravel().astype(np.int64)
    estart = estart.ravel()
    core_s = np.arange(s_total, dtype=np.int64) // spc

    order_e = np.argsort(seg, kind="stable")
    seg_sorted = seg[order_e]
    seg_start = np.zeros(s_total + 1, np.int64)
    np.cumsum(counts, out=seg_start[1:])
    within = np.arange(e_total, dtype=np.int64) - seg_start[seg_sorted]

    slot = estart[seg_sorted] + within          # edge slot within block [0, cap)
    t_e = slot // 128
    p_e = slot % 128
    b_e = binid[seg_sorted]
    c_e = core_s[seg_sorted]

    nbrows = nb * k * 128                        # stream rows per core
    row = ((b_e // bps) * 128 + p_e) * (bps * k) + (b_e % bps) * k + t_e
    grow = c_e * nbrows + row

    vec16 = np.ascontiguousarray(edge_vec, dtype=np.float16)
    stream = np.zeros((ncores * nbrows, d), np.float16)
    stream[grow] = vec16[order_e]

    # seg_rel table: [ncores, 128(part), nb*k]; PAD for unused edge slots.
    segrel = np.full(ncores * 128 * nb * k, PAD_SEGREL, np.float16)
    segrel[(c_e * 128 + p_e) * (nb * k) + b_e * k + t_e] = slocal[seg_sorted]
    segrel = segrel.reshape(ncores, 128, nb * k)

    # 1/count per (core, psum partition, block pair); 1.0 for empty slots.
    # Partition row = slocal + 64 * (block parity).
    npair = nb // 2
    invc = np.ones(ncores * 128 * npair, np.float32)
    invc[
        (core_s * 128 + slocal + G * (binid % 2)) * npair + binid // 2
    ] = 1.0 / np.maximum(counts, 1)
    invc = invc.reshape(ncores, 128, npair)

    iota = np.broadcast_to(np.arange(G, dtype=np.float16), (128, G)).copy()

    # Inverse map into the partition-major device output layout
    # [nsb, 128, bps//2, d]: segment (binid, slocal) lives at super-block
    # isb = binid//bps, partition (binid%2)*G + slocal, pair j = (binid%bps)//2.
    isb_s = binid // bps
    j_s = (binid % bps) // 2
    q_s = binid % 2
    seg_of = np.full(ncores * nb * G, -1, np.int64)
    seg_of[
        core_s * (nb * G)
        + ((isb_s * 128 + q_s * G + slocal) * (bps // 2) + j_s)
    ] = np.arange(s_total)
    seg_of = seg_of.reshape(ncores, nb * G)

    return nb, stream, segrel, invc, iota, seg_of


def _build_graph(nb, k, bps, d):
    import concourse.tile as tile
    from concourse import bacc, mybir

    f16 = mybir.dt.float16
    f32 = mybir.dt.float32
    nsb = nb // bps
    bk = bps * k            # vec tiles per super-block (24)
    npairs = bps // 2       # psum column groups per super-block (4)

    nc = bacc.Bacc()
    vec_p = nc.declare_dram_parameter("vec", [nsb * 128, bk * 128], f16, isOutput=False)
    srel_p = nc.declare_dram_parameter("srel", [128, nb * k], f16, isOutput=False)
    invc_p = nc.declare_dram_parameter("invc", [128, nb // 2], f32, isOutput=False)
    iota_p = nc.declare_dram_parameter("iota", [128, G], f16, isOutput=False)
    out_p = nc.declare_dram_parameter(
        "out", [nsb * 128, npairs * d], f16, isOutput=True
    )

    with tile.TileContext(nc) as tc:
        with tc.tile_pool(name="const", bufs=1) as cpool, \
             tc.tile_pool(name="vecp", bufs=4) as vpool, \
             tc.tile_pool(name="ohp", bufs=4) as opool, \
             tc.tile_pool(name="resp", bufs=4) as rpool, \
             tc.tile_pool(name="psp", bufs=4, space="PSUM") as ppool:

            srel_t = cpool.tile([128, nb * k], f16)
            nc.sync.dma_start(out=srel_t[:], in_=srel_p[:, :])
            invc_t = cpool.tile([128, nb // 2], f32)
            nc.sync.dma_start(out=invc_t[:], in_=invc_p[:, :])
            iota_t = cpool.tile([128, G], f16)
            nc.sync.dma_start(out=iota_t[:], in_=iota_p[:, :])

            for isb in range(nsb):
                vt = vpool.tile([128, bk * 128], f16)
                dma_eng = nc.sync if isb % 2 == 0 else nc.scalar
                dma_eng.dma_start(
                    out=vt[:], in_=vec_p[isb * 128 : (isb + 1) * 128, :]
                )
                oh = opool.tile([128, bk * G], f16)
                nc.vector.tensor_tensor(
                    out=oh[:].rearrange("p (a s) -> p a s", s=G),
                    in0=srel_t[:, isb * bk : (isb + 1) * bk]
                    .unsqueeze(2)
                    .to_broadcast([128, bk, G]),
                    in1=iota_t[:].unsqueeze(1).to_broadcast([128, bk, G]),
                    op=mybir.AluOpType.is_equal,
                )
                ps = ppool.tile([128, npairs * 128], f32, space="PSUM")
                for j in range(npairs):
                    for q in range(2):
                        for t in range(k):
                            tau = (j * 2 + q) * k + t
                            nc.tensor.matmul(
                                out=ps[q * G : (q + 1) * G, j * 128 : (j + 1) * 128],
                                lhsT=oh[:, tau * G : (tau + 1) * G],
                                rhs=vt[:, tau * 128 : (tau + 1) * 128],
                                start=(t == 0),
                                stop=(t == k - 1),
                            )
                ot = rpool.tile([128, npairs * d], f16)
                nc.vector.tensor_tensor(
                    out=ot[:].rearrange("p (a s) -> p a s", s=d),
                    in0=ps[:].rearrange("p (a s) -> p a s", s=128),
                    in1=invc_t[:, isb * npairs : (isb + 1) * npairs]
                    .unsqueeze(2)
                    .to_broadcast([128, npairs, 128]),
                    op=mybir.AluOpType.mult,
                )
                nc.scalar.dma_start(
                    out=out_p[isb * 128 : (isb + 1) * 128, :],
                    in_=ot[:],
                )
    nc.compile()
    return nc


def _run(edge_vec, seg, s_total=S, ncores=NCORES, k=K, bps=BPS, trace=False):
    from concourse.bass_utils import run_bass_kernel_spmd

    edge_vec = np.asarray(edge_vec, dtype=np.float32)
    seg = np.asarray(seg, dtype=np.int64)
    d = edge_vec.shape[1]

    nb, stream, segrel, invc, iota, seg_of = _prepare(
        edge_vec, seg, s_total, ncores, k, bps
    )
    nc = _build_graph(nb, k, bps, d)

    nbrows = nb * k * 128
    in_maps = [
        {
            "vec": stream[c * nbrows : (c + 1) * nbrows].reshape(
                (nb // bps) * 128, bps * k * 128
            ),
            "srel": segrel[c],
            "invc": invc[c],
            "iota": iota,
        }
        for c in range(ncores)
    ]
    res = run_bass_kernel_spmd(
        nc, in_maps, core_ids=list(range(ncores)), trace=trace
    )

    dev = np.concatenate(
        [res.results[c]["out"].reshape(-1, d) for c in range(ncores)], axis=0
    )
    out_flat = np.zeros((s_total, d), np.float32)
    mask = seg_of.ravel() >= 0
    out_flat[seg_of.ravel()[mask]] = dev[mask].astype(np.float32)
    return out_flat, res, nc


def kernel(edge_vec, selected_edges, num_segments=S, batch_size=B, n_nodes=N):
    selected_edges = np.asarray(selected_edges)
    seg = np.asarray(selected_edges[:, 5], dtype=np.int64)
    s_total = int(num_segments)
    out_flat, _, _ = _run(edge_vec, seg, s_total=s_total)
    return out_flat.reshape(int(batch_size), int(n_nodes), -1)
